# revision 44
# baseline (speedup 1.0000x reference)
"""Causal single-head attention (B=4, S=2048, D=1024) on 8 Trainium2 cores.

Sharding: 8 cores = (batch b, stripe-set eta). Core (b, eta) owns eight
interleaved key stripes of 128 rows at global offsets 256k + 128*eta
(k = 0..7) of batch b, stored locally stripe-major. Queries are fed
"aligned" with base beta = 128*eta: query col c corresponds to global row
beta + c. Then the causal condition for key tile kt (= stripe kt) vs
query chunk rc is c >= 256*kt + x - identical on every core, so one SPMD
program serves both stripe sets with a purely compile-time block mask;
score blocks with kt >= 2*(rc+1) are skipped outright and boundary tiles
are trimmed 256 cols, giving tile-exact causality. Cols past the
sequence end (eta=1, c >= 1920) compute junk that the host discards.

Softmax uses no max-subtraction (logits are O(1): |score/32| < ~4), so
per-core partials are num = exp(S)*V and l = sum(exp(S)), both carrying
a common 1/4 prescale (EXP_BIAS) so the numerator ships as fp16 with 4x
headroom; the host merges halves with num/den addition and one divide.

The Q and K projections are folded away algebraically: scores =
x_kv (Wk^T Wq) x^T with G = Wk^T Wq precomputed on the host. On-chip:
M^T = G^T x_kv^T costs 1024*D^2 MACs, replacing the 2048*D^2 Q
projection and 1024*D^2 K projection outright.

Precision split (validated vs the fp32 reference on the host: rel err
1.3e-2 < 2e-2 gate): the two score-side contractions (MT = G^T x_kv^T
and ST = MT^T x^T) run in fp8 e4m3 with DoubleRow perf mode - two
128-row contraction blocks per pass, ~1.8x the fp16 matmul rate. The
value path (V = x_kv^T Wv^T and PV) stays fp16: quantizing it leaks
fp8 noise directly into the output. Power-of-two prescales keep every
fp8 operand in e4m3's sweet spot (x*32, G*2048, M*2^-10 => 64*M) and
are folded exactly into the exp activation scale 2^-16.

On-chip layout: fp8 operands are [128, 8, N] tiles (dim1 = contraction
block) so a DoubleRow matmul consumes [:, 2k:2k+2, cols] directly.
    MT = g.T @ xkv   (fp8 DR)      V = xkv.T @ wvT   (fp16)
    ST = MT.T @ xt   (fp8 DR, scores transposed: partition=key)
    PT = exp(ST*2^-16) causally zeroed, stored fp16. PV runs with PT
    sub-blocks stationary and V moving; the denominator comes free as
    an N=1 matmul on the same stationary: l = PT_sub.T @ 1s.
Emission: warmup (HAM un-throttle, covers the DMA-trigger preamble) ->
MT -> V -> ST(0..7) -> PV(3),PV(2),PV(1),PV(0) so the kernel ends on
the *smallest* PV chunk and the final output DMA is tiny.
Outputs per core: ot [NQ, D] fp16 normalized, ls [128, 16] fp32 denom
(query col c lives at ls[c % 128, c // 128]).
"""

import sys

sys.path.insert(0, "/opt/trn_rl_repo")

from contextlib import ExitStack

import ml_dtypes
import numpy as np

import concourse.bass as bass  # noqa: F401  (engine types resolve via bacc)
import concourse.mybir as mybir
import concourse.tile as tile
from concourse import bacc, bass_utils
from concourse.bass import ts

F16 = mybir.dt.float16
F32 = mybir.dt.float32
F8 = mybir.dt.float8e4
DR = mybir.MatmulPerfMode.DoubleRow

P = 128            # partitions
D = 1024           # model dim (d_in == d_out)
NQ = 2048          # query slots per core
NK = 1024          # keys per core
RC = 512           # query-chunk (matmul moving-dim) size
N_RC = NQ // RC    # 4
N_KT = NK // P     # 8 key tiles
N_IB = D // P      # 8 contraction blocks
N_KP = N_IB // 2   # 4 DoubleRow contraction pairs

# power-of-two fp8 prescales; the exp scale folds them all back out:
# mt8 = psum copy of (32 G)*(1 x) = 32*m (|max| ~59 << 240, no scale op
# needed), st_psum = (32 m)*(1 x) = 32*S, logits = S/32 => 2^-10.
# The -ln(4) bias scales num AND den by 1/4 (ratio exact) so the fp16
# numerator ships with 4x headroom (|num| < ~1.5K vs 65504).
X_SCALE = 1.0
G_SCALE = 32.0
EXP_SCALE = 2.0 ** -10
EXP_BIAS = -1.3862943611198906  # -ln(4)

N_CORES = 8
B, S = 4, 2048
STRIPE = 128


def _kept_kts(rc):
    # key tile kt (= stripe kt, 128 keys at global 256*kt + 128*eta) is
    # visible to query chunk rc iff rc*512 + 511 >= 256*kt.
    return [kt for kt in range(N_KT) if kt < 2 * (rc + 1)]


def _mask_base(rc, kt):
    # stripe width 128: key tile kt IS stripe kt, threshold c >= 256*kt + x
    return RC * rc - 2 * P * kt


def _trim(rc, kt):
    # boundary tile kt == 2rc+1: its first 256 query cols lie strictly
    # below the causal diagonal - skip them entirely.
    return 2 * P if kt == 2 * rc + 1 else 0


def _emit(nc, tc, xt8, kv8a, kv8b, g8, kv16, wva, wvb, ot, ls):
    with ExitStack() as ctx:
        sb = ctx.enter_context(tc.tile_pool(name="sb", bufs=1))
        pts = ctx.enter_context(tc.tile_pool(name="pts", bufs=1))
        outp = ctx.enter_context(tc.tile_pool(name="outp", bufs=8))
        ps = ctx.enter_context(tc.tile_pool(name="ps", bufs=8, space="PSUM"))

        # warm is the PE warmup's only dependency - memset it first
        warm = sb.tile([P, P], F16, tag="warm", name="warm")
        nc.vector.memset(warm, 0.0)
        ones = sb.tile([P, 1], F16, tag="ones", name="ones")
        nc.vector.memset(ones, 1.0)
        ebias = sb.tile([P, 1], F32, tag="ebias", name="ebias")
        nc.vector.memset(ebias, EXP_BIAS)

        # HAM warm-up: dummy matmuls needing no DMA, issued while the NEFF
        # preamble + first input loads run. They lift the PE clock gate
        # from 1.2 to 2.4 GHz before real matmuls arrive. N=128 keeps the
        # end-granularity fine. Parked in l_sb (every column overwritten).
        l_sb = sb.tile([P, N_RC * 4], F32, tag="lsb", name="lsb")
        acc_w = ps.tile([P, P], F32, tag="mm", name="acc_w")
        N_WARM = 50
        for w in range(N_WARM):
            nc.tensor.matmul(acc_w, lhsT=warm, rhs=warm,
                             start=(w == 0), stop=(w == N_WARM - 1))
        nc.vector.tensor_copy(l_sb, acc_w[:, 0:N_RC * 4])

        # ---- input loads ----
        # Emission order = consumption order, and operands are column-
        # split by consuming sub-phase (kv8 by jc, wv by dc) so each
        # phase's first batch waits on the minimum possible bytes:
        # MT-jc0 needs just g8+kv8a (1.5MB), V-dc0 just kv16+wva.
        xt8_sb = sb.tile([P, N_IB, NQ], F8, tag="xt8", name="xt8_sb")
        kv8a_sb = sb.tile([P, N_IB, RC], F8, tag="kv8a", name="kv8a_sb")
        kv8b_sb = sb.tile([P, N_IB, RC], F8, tag="kv8b", name="kv8b_sb")
        g8_sb = sb.tile([P, N_IB, D], F8, tag="g8", name="g8_sb")
        # fp16 operands stay 2D: 3D-sliced APs defeat the LDWEIGHTS
        # pull-ahead and cost ~43ns per matmul (measured). Two contraction
        # blocks ride in one 2D tile so each load is a single trigger
        # (sync-queue triggers cost ~640ns each, and V waits on the last).
        kv16_p = [sb.tile([P, 2 * NK], F16, tag=f"kv16_{h}", name=f"kv16_{h}")
                  for h in range(N_KP)]
        wva_p = [sb.tile([P, 2 * RC], F16, tag=f"wva{h}", name=f"wva{h}")
                 for h in range(N_KP)]
        wvb_p = [sb.tile([P, 2 * RC], F16, tag=f"wvb{h}", name=f"wvb{h}")
                 for h in range(N_KP)]

        def kv16_at(i):
            return kv16_p[i // 2][:, (i % 2) * NK:(i % 2) * NK + NK]

        def wv_at(i, dc):
            p = (wva_p if dc == 0 else wvb_p)[i // 2]
            return p[:, (i % 2) * RC:(i % 2) * RC + RC]

        for k in range(N_KP):
            nc.sync.dma_start(out=g8_sb[:, 2 * k:2 * k + 2, :],
                              in_=g8[:, 2 * k:2 * k + 2, :])
            nc.sync.dma_start(out=kv8a_sb[:, 2 * k:2 * k + 2, :],
                              in_=kv8a[:, 2 * k:2 * k + 2, :])
        for h in range(2):
            nc.sync.dma_start(out=kv8b_sb[:, 4 * h:4 * h + 4, :],
                              in_=kv8b[:, 4 * h:4 * h + 4, :])
        for k in range(N_KP):
            nc.sync.dma_start(out=kv16_p[k], in_=kv16[:, 2 * k:2 * k + 2, :])
        for k in range(N_KP):
            nc.sync.dma_start(out=wva_p[k], in_=wva[:, 2 * k:2 * k + 2, :])
        for k in range(N_KP):
            nc.sync.dma_start(out=wvb_p[k], in_=wvb[:, 2 * k:2 * k + 2, :])
        for h in range(2):
            nc.sync.dma_start(out=xt8_sb[:, 4 * h:4 * h + 4, :],
                              in_=xt8[:, 4 * h:4 * h + 4, :])

        # ---- MT projection (fp8 DoubleRow) ----
        # mt8[p, o, j] = sum_i g[i, o*128+p] xkv[i, j], scaled to 32*m.
        # Four sub-phases of 4 PSUM groups; contraction k-pairs stream in
        # DMA-arrival order, and each sub-phase's casts (split across both
        # elementwise engines, ~680ns apiece) hide under the next one.
        mt8_sb = sb.tile([P, N_IB, NK], F8, tag="mt8", name="mt8_sb")
        for jc in range(NK // RC):
            for oh in range(2):
                accs = [(oh * 4 + i, ps.tile([P, RC], F32, tag="mm",
                                             name="acc_mt"))
                        for i in range(4)]
                kv8_h = kv8a_sb if jc == 0 else kv8b_sb
                for k in range(N_KP):
                    for o, a in accs:
                        nc.tensor.matmul(
                            a, lhsT=g8_sb[:, 2 * k:2 * k + 2, ts(o, P)],
                            rhs=kv8_h[:, 2 * k:2 * k + 2, :],
                            start=(k == 0), stop=(k == N_KP - 1),
                            perf_mode=DR)
                        if k == N_KP - 1:
                            if o % 2 == 0:
                                nc.vector.tensor_copy(
                                    mt8_sb[:, o, ts(jc, RC)], a)
                            else:
                                nc.scalar.copy(mt8_sb[:, o, ts(jc, RC)], a)

        # ---- V projection (fp16) ----
        # dc-major batches: the dc=0 half only needs wva, which arrives
        # a full load-phase before wvb
        v_sb = [sb.tile([P, D], F16, tag=f"vj{j}", name=f"vj{j}")
                for j in range(N_KT)]
        groups = [(j, dc) for dc in range(D // RC) for j in range(N_KT)]
        for gb in range(0, len(groups), 4):
            batch = groups[gb:gb + 4]
            accs = [ps.tile([P, RC], F32, tag="mm", name="acc_v")
                    for _ in batch]
            for i in range(N_IB):
                for a, (j, dc) in zip(accs, batch):
                    nc.tensor.matmul(a, lhsT=kv16_at(i)[:, ts(j, P)],
                                     rhs=wv_at(i, dc),
                                     start=(i == 0), stop=(i == N_IB - 1))
            for n, (a, (j, dc)) in enumerate(zip(accs, batch)):
                if n % 2 == 0:
                    nc.vector.tensor_copy(v_sb[j][:, ts(dc, RC)], a)
                else:
                    nc.scalar.copy(v_sb[j][:, ts(dc, RC)], a)

        # ---- attention scores (fp8 DoubleRow) ----
        # ST is emitted kt-major so the stationary MT block is reused by
        # consecutive matmuls across query chunks.
        pt_tiles = {}

        def kept_rcs(kt):
            return [rc for rc in range(N_RC) if kt in _kept_kts(rc)]

        def emit_st(kt):
            rcs = kept_rcs(kt)
            accs = {rc: ps.tile([P, RC], F32, tag="mm", name="acc_st")
                    for rc in rcs}
            for k in range(N_KP):
                for rc in rcs:
                    qo = _trim(rc, kt)
                    nc.tensor.matmul(
                        accs[rc][:, qo:RC],
                        lhsT=mt8_sb[:, 2 * k:2 * k + 2, ts(kt, P)],
                        rhs=xt8_sb[:, 2 * k:2 * k + 2,
                                   rc * RC + qo:(rc + 1) * RC],
                        start=(k == 0), stop=(k == N_KP - 1), perf_mode=DR)
            for rc in rcs:
                qo = _trim(rc, kt)
                pt = pts.tile([P, RC], F16, tag=f"pt{kt}_{rc}",
                              name=f"pt{kt}_{rc}")
                nc.scalar.activation(pt[:, qo:RC], accs[rc][:, qo:RC],
                                     mybir.ActivationFunctionType.Exp,
                                     bias=ebias, scale=EXP_SCALE)
                base = _mask_base(rc, kt) + qo
                if base < P - 1:  # tile straddles the causal diagonal
                    nc.gpsimd.affine_select(
                        out=pt[:, qo:RC], in_=pt[:, qo:RC],
                        compare_op=mybir.AluOpType.is_ge, fill=0.0,
                        base=base, channel_multiplier=-1,
                        pattern=[[1, RC - qo]])
                pt_tiles[(kt, rc)] = pt

        # ---- PV (fp16) ----
        # PT sub-blocks stationary, V moving; output lands in natural
        # [query, d] orientation; the row-sum l is an extra N=1 matmul on
        # an already-loaded stationary. Numerators ship fp16.
        def emit_pv(rc, rsub):
            # the trimmed sub-block (kt == 2rc+1, rsub < 2) is all-zero
            kts = [kt for kt in _kept_kts(rc)
                   if not (rsub < _trim(rc, kt) // P)]
            last = len(kts) - 1
            pos = [ps.tile([P, RC], F32, tag="mm", name="acc_pv")
                   for _ in range(D // RC)]
            pl = ps.tile([P, 1], F32, tag="mm", name="acc_l")
            for n, kt in enumerate(kts):
                lhs = pt_tiles[(kt, rc)][:, ts(rsub, P)]
                for dc, po in enumerate(pos):
                    nc.tensor.matmul(po, lhsT=lhs,
                                     rhs=v_sb[kt][:, ts(dc, RC)],
                                     start=(n == 0), stop=(n == last))
                nc.tensor.matmul(pl, lhsT=lhs, rhs=ones,
                                 start=(n == 0), stop=(n == last))
            # NOTE: do NOT widen ot rows past 1024 f16 cols - a 2052-byte
            # row defeats DMA burst aggregation (one packet per row,
            # ~75GB/s instead of ~200) and adds ~20us of output tail.
            idx = rc * 4 + rsub
            nc.vector.tensor_copy(l_sb[:, idx:idx + 1], pl)
            o_sb = outp.tile([P, D], F16, tag="osb", name="osb")
            nc.vector.tensor_copy(o_sb[:, ts(0, RC)], pos[0])
            nc.scalar.copy(o_sb[:, ts(1, RC)], pos[1])
            row = rc * RC + rsub * P
            # split by partition halves: a [128, D] transfer is 128
            # per-partition packets on ONE queue (~3.8us); halves ride
            # two queues in parallel
            nc.sync.dma_start(out=ot[row:row + 64, :], in_=o_sb[0:64, :])
            nc.sync.dma_start(out=ot[row + 64:row + P, :], in_=o_sb[64:P, :])

        # thick/thin interleave keeps at most 5 PSUM slots per kt-pair in
        # flight (vs 8 for 0,1,2,...) so the scalar exp chain never gates
        # the ring
        for kt in (0, 7, 1, 6, 2, 5, 3, 4):
            emit_st(kt)
        # pair thin (rc 0/1) with thick (rc 2/3) rsubs so the elementwise
        # engines' output casts always hide under PE compute, and end on
        # the deepest chunk (3,3) so the kernel tail is one cast + DMA
        # NOTE: gpsimd.dma_start (SWDGE) costs ~43ns on EVERY matmul in the
        # module - never add a dynamic DMA queue to this kernel.
        for rc_pair in ((0, 2), (1, 3)):
            for rsub in range(RC // P):
                for rc in rc_pair:
                    emit_pv(rc, rsub)
        nc.sync.dma_start(out=ls, in_=l_sb)


_NC_CACHE = {}


def _get_nc():
    if "nc" not in _NC_CACHE:
        nc = bacc.Bacc("TRN2", target_bir_lowering=False, debug=False,
                       enable_asserts=False, num_devices=N_CORES)
        xt8 = nc.dram_tensor("xt8", [P, N_IB, NQ], F8, kind="ExternalInput").ap()
        kv8a = nc.dram_tensor("kv8a", [P, N_IB, RC], F8,
                              kind="ExternalInput").ap()
        kv8b = nc.dram_tensor("kv8b", [P, N_IB, RC], F8,
                              kind="ExternalInput").ap()
        g8 = nc.dram_tensor("g8", [P, N_IB, D], F8, kind="ExternalInput").ap()
        kv16 = nc.dram_tensor("kv16", [P, N_IB, NK], F16,
                              kind="ExternalInput").ap()
        wva = nc.dram_tensor("wva", [P, N_IB, RC], F16,
                             kind="ExternalInput").ap()
        wvb = nc.dram_tensor("wvb", [P, N_IB, RC], F16,
                             kind="ExternalInput").ap()
        ot = nc.dram_tensor("ot", [NQ, D], F16, kind="ExternalOutput").ap()
        ls = nc.dram_tensor("ls", [P, N_RC * 4], F32, kind="ExternalOutput").ap()
        with tile.TileContext(nc) as tc:
            _emit(nc, tc, xt8, kv8a, kv8b, g8, kv16, wva, wvb, ot, ls)
        nc.compile()
        _NC_CACHE["nc"] = nc
    return _NC_CACHE["nc"]


def _blk(a, width):
    # [D, width] row-major -> [128, 8, width] (dim1 = 128-row block)
    return np.ascontiguousarray(
        a.reshape(N_IB, P, width).transpose(1, 0, 2))


def _f8(a, scale):
    return np.asarray(np.clip(a * scale, -240.0, 240.0),
                      dtype=ml_dtypes.float8_e4m3)


def make_in_maps(x, w_query, w_key, w_value):
    wq32 = np.asarray(w_query, dtype=np.float32)
    wk32 = np.asarray(w_key, dtype=np.float32)
    # fold the Q and K projections: scores = x_kv (Wk^T Wq) x^T
    g_np = np.ascontiguousarray(wk32.T @ wq32)
    g8_np = _blk(_f8(g_np, G_SCALE), D)
    wvt_np = np.ascontiguousarray(np.asarray(w_value).T).astype(np.float16)
    wva_np = _blk(np.ascontiguousarray(wvt_np[:, 0:RC]), RC)
    wvb_np = _blk(np.ascontiguousarray(wvt_np[:, RC:D]), RC)
    kv_cols = (np.arange(NK) // STRIPE) * (2 * STRIPE) + np.arange(NK) % STRIPE
    in_maps = []
    for c in range(N_CORES):
        b, eta = c // 2, c % 2
        rows = (np.arange(NQ) + eta * STRIPE) % S  # cols past S wrap to junk
        xt_np = np.ascontiguousarray(np.asarray(x)[b, rows].T)  # [D, NQ] f32
        xkv_np = xt_np[:, kv_cols]                              # [D, NK] f32
        kv8_np = _f8(xkv_np, X_SCALE)
        in_maps.append({
            "xt8": _blk(_f8(xt_np, X_SCALE), NQ),
            "kv8a": _blk(np.ascontiguousarray(kv8_np[:, 0:RC]), RC),
            "kv8b": _blk(np.ascontiguousarray(kv8_np[:, RC:NK]), RC),
            "g8": g8_np,
            "kv16": _blk(xkv_np.astype(np.float16), NK),
            "wva": wva_np,
            "wvb": wvb_np,
        })
    return in_maps


def merge_outputs(results):
    num = np.zeros((B, S, D), np.float32)
    den = np.zeros((B, S), np.float32)
    for c in range(N_CORES):
        b, eta = c // 2, c % 2
        # ot is the fp16 numerator, ls the denominator (both carry the
        # common 1/4 prescale from EXP_BIAS; the ratio is exact)
        otc = np.asarray(results[c]["ot"]).astype(np.float32)  # [NQ, D]
        # ls[p, col] holds l for query col c = col*128 + p
        lc = np.asarray(results[c]["ls"]).T.reshape(NQ)
        beta = eta * STRIPE
        nvalid = S - beta
        num[b, beta:] += otc[:nvalid]
        den[b, beta:] += lc[:nvalid]
    return (num / den[:, :, None]).astype(np.float32)


def kernel(x, w_query, w_key, w_value, _trace=False):
    nc = _get_nc()
    in_maps = make_in_maps(x, w_query, w_key, w_value)
    res = bass_utils.run_bass_kernel_spmd(
        nc, in_maps, core_ids=list(range(N_CORES)), trace=_trace)
    out = merge_outputs(res.results)
    if _trace:
        kernel.last_result = res
    return out


# revision 48
# speedup vs baseline: 1.0056x; 1.0056x over previous
"""Causal single-head attention (B=4, S=2048, D=1024) on 8 Trainium2 cores.

Sharding: 8 cores = (batch b, stripe-set eta). Core (b, eta) owns eight
interleaved key stripes of 128 rows at global offsets 256k + 128*eta
(k = 0..7) of batch b, stored locally stripe-major. Queries are fed
"aligned" with base beta = 128*eta: query col c corresponds to global row
beta + c. Then the causal condition for key tile kt (= stripe kt) vs
query chunk rc is c >= 256*kt + x - identical on every core, so one SPMD
program serves both stripe sets with a purely compile-time block mask;
score blocks with kt >= 2*(rc+1) are skipped outright and boundary tiles
are trimmed 256 cols, giving tile-exact causality. Cols past the
sequence end (eta=1, c >= 1920) compute junk that the host discards.

Softmax uses no max-subtraction (logits are O(1): |score/32| < ~4), so
per-core partials are num = exp(S)*V and l = sum(exp(S)), both carrying
a common 1/4 prescale (EXP_BIAS) so the numerator ships as fp16 with 4x
headroom; the host merges halves with num/den addition and one divide.

The Q and K projections are folded away algebraically: scores =
x_kv (Wk^T Wq) x^T with G = Wk^T Wq precomputed on the host. On-chip:
M^T = G^T x_kv^T costs 1024*D^2 MACs, replacing the 2048*D^2 Q
projection and 1024*D^2 K projection outright.

Precision split (validated vs the fp32 reference on the host: rel err
1.3e-2 < 2e-2 gate): the two score-side contractions (MT = G^T x_kv^T
and ST = MT^T x^T) run in fp8 e4m3 with DoubleRow perf mode - two
128-row contraction blocks per pass, ~1.8x the fp16 matmul rate. The
value path (V = x_kv^T Wv^T and PV) stays fp16: quantizing it leaks
fp8 noise directly into the output. Power-of-two prescales keep every
fp8 operand in e4m3's sweet spot (x*32, G*2048, M*2^-10 => 64*M) and
are folded exactly into the exp activation scale 2^-16.

On-chip layout: fp8 operands are [128, 8, N] tiles (dim1 = contraction
block) so a DoubleRow matmul consumes [:, 2k:2k+2, cols] directly.
    MT = g.T @ xkv   (fp8 DR)      V = xkv.T @ wvT   (fp16)
    ST = MT.T @ xt   (fp8 DR, scores transposed: partition=key)
    PT = exp(ST*2^-16) causally zeroed, stored fp16. PV runs with PT
    sub-blocks stationary and V moving; the denominator comes free as
    an N=1 matmul on the same stationary: l = PT_sub.T @ 1s.
Emission: warmup (HAM un-throttle, covers the DMA-trigger preamble) ->
MT -> V -> ST(0..7) -> PV(3),PV(2),PV(1),PV(0) so the kernel ends on
the *smallest* PV chunk and the final output DMA is tiny.
Outputs per core: ot [NQ, D] fp16 normalized, ls [128, 16] fp32 denom
(query col c lives at ls[c % 128, c // 128]).
"""

import sys

sys.path.insert(0, "/opt/trn_rl_repo")

from contextlib import ExitStack

import ml_dtypes
import numpy as np

import concourse.bass as bass  # noqa: F401  (engine types resolve via bacc)
import concourse.mybir as mybir
import concourse.tile as tile
from concourse import bacc, bass_utils
from concourse.bass import ts

F16 = mybir.dt.float16
F32 = mybir.dt.float32
F8 = mybir.dt.float8e4
DR = mybir.MatmulPerfMode.DoubleRow

P = 128            # partitions
D = 1024           # model dim (d_in == d_out)
NQ = 2048          # query slots per core
NK = 1024          # keys per core
RC = 512           # query-chunk (matmul moving-dim) size
N_RC = NQ // RC    # 4
N_KT = NK // P     # 8 key tiles
N_IB = D // P      # 8 contraction blocks
N_KP = N_IB // 2   # 4 DoubleRow contraction pairs

# power-of-two fp8 prescales; the exp scale folds them all back out:
# mt8 = psum copy of (32 G)*(1 x) = 32*m (|max| ~59 << 240, no scale op
# needed), st_psum = (32 m)*(1 x) = 32*S, logits = S/32 => 2^-10.
# The -ln(4) bias scales num AND den by 1/4 (ratio exact) so the fp16
# numerator ships with 4x headroom (|num| < ~1.5K vs 65504).
X_SCALE = 1.0
G_SCALE = 32.0
EXP_SCALE = 2.0 ** -10
EXP_BIAS = -1.3862943611198906  # -ln(4)

N_CORES = 8
B, S = 4, 2048
STRIPE = 128


def _kept_kts(rc):
    # key tile kt (= stripe kt, 128 keys at global 256*kt + 128*eta) is
    # visible to query chunk rc iff rc*512 + 511 >= 256*kt.
    return [kt for kt in range(N_KT) if kt < 2 * (rc + 1)]


def _mask_base(rc, kt):
    # stripe width 128: key tile kt IS stripe kt, threshold c >= 256*kt + x
    return RC * rc - 2 * P * kt


def _trim(rc, kt):
    # boundary tile kt == 2rc+1: its first 256 query cols lie strictly
    # below the causal diagonal - skip them entirely.
    return 2 * P if kt == 2 * rc + 1 else 0


def _emit(nc, tc, xt8, kv8a, kv8b, g8, kv16, wva, wvb, ot, ls):
    with ExitStack() as ctx:
        sb = ctx.enter_context(tc.tile_pool(name="sb", bufs=1))
        pts = ctx.enter_context(tc.tile_pool(name="pts", bufs=1))
        outp = ctx.enter_context(tc.tile_pool(name="outp", bufs=8))
        ps = ctx.enter_context(tc.tile_pool(name="ps", bufs=8, space="PSUM"))

        # warm is the PE warmup's only dependency - memset it first
        warm = sb.tile([P, P], F16, tag="warm", name="warm")
        nc.vector.memset(warm, 0.0)
        ones = sb.tile([P, 1], F16, tag="ones", name="ones")
        nc.vector.memset(ones, 1.0)
        ebias = sb.tile([P, 1], F32, tag="ebias", name="ebias")
        nc.vector.memset(ebias, EXP_BIAS)

        # HAM warm-up: dummy matmuls needing no DMA, issued while the NEFF
        # preamble + first input loads run. They lift the PE clock gate
        # from 1.2 to 2.4 GHz before real matmuls arrive. N=128 keeps the
        # end-granularity fine. Parked in l_sb (every column overwritten).
        l_sb = sb.tile([P, N_RC * 4], F32, tag="lsb", name="lsb")
        acc_w = ps.tile([P, P], F32, tag="mm", name="acc_w")
        N_WARM = 50
        for w in range(N_WARM):
            nc.tensor.matmul(acc_w, lhsT=warm, rhs=warm,
                             start=(w == 0), stop=(w == N_WARM - 1))
        nc.vector.tensor_copy(l_sb, acc_w[:, 0:N_RC * 4])

        # ---- input loads ----
        # Emission order = consumption order, and operands are column-
        # split by consuming sub-phase (kv8 by jc, wv by dc) so each
        # phase's first batch waits on the minimum possible bytes:
        # MT-jc0 needs just g8+kv8a (1.5MB), V-dc0 just kv16+wva.
        xt8_sb = sb.tile([P, N_IB, NQ], F8, tag="xt8", name="xt8_sb")
        kv8a_sb = sb.tile([P, N_IB, RC], F8, tag="kv8a", name="kv8a_sb")
        kv8b_sb = sb.tile([P, N_IB, RC], F8, tag="kv8b", name="kv8b_sb")
        g8_sb = sb.tile([P, N_IB, D], F8, tag="g8", name="g8_sb")
        # fp16 operands stay 2D: 3D-sliced APs defeat the LDWEIGHTS
        # pull-ahead and cost ~43ns per matmul (measured). Two contraction
        # blocks ride in one 2D tile so each load is a single trigger
        # (sync-queue triggers cost ~640ns each, and V waits on the last).
        kv16_p = [sb.tile([P, 2 * NK], F16, tag=f"kv16_{h}", name=f"kv16_{h}")
                  for h in range(N_KP)]
        wva_p = [sb.tile([P, 2 * RC], F16, tag=f"wva{h}", name=f"wva{h}")
                 for h in range(N_KP)]
        wvb_p = [sb.tile([P, 2 * RC], F16, tag=f"wvb{h}", name=f"wvb{h}")
                 for h in range(N_KP)]

        def kv16_at(i):
            return kv16_p[i // 2][:, (i % 2) * NK:(i % 2) * NK + NK]

        def wv_at(i, dc):
            p = (wva_p if dc == 0 else wvb_p)[i // 2]
            return p[:, (i % 2) * RC:(i % 2) * RC + RC]

        for k in range(N_KP):
            nc.sync.dma_start(out=g8_sb[:, 2 * k:2 * k + 2, :],
                              in_=g8[:, 2 * k:2 * k + 2, :])
            nc.sync.dma_start(out=kv8a_sb[:, 2 * k:2 * k + 2, :],
                              in_=kv8a[:, 2 * k:2 * k + 2, :])
        for h in range(2):
            nc.sync.dma_start(out=kv8b_sb[:, 4 * h:4 * h + 4, :],
                              in_=kv8b[:, 4 * h:4 * h + 4, :])
        for k in range(N_KP):
            nc.sync.dma_start(out=kv16_p[k], in_=kv16[:, 2 * k:2 * k + 2, :])
        for k in range(N_KP):
            nc.sync.dma_start(out=wva_p[k], in_=wva[:, 2 * k:2 * k + 2, :])
        for k in range(N_KP):
            nc.sync.dma_start(out=wvb_p[k], in_=wvb[:, 2 * k:2 * k + 2, :])
        for h in range(2):
            nc.sync.dma_start(out=xt8_sb[:, 4 * h:4 * h + 4, :],
                              in_=xt8[:, 4 * h:4 * h + 4, :])

        # ---- MT projection (fp8 DoubleRow) ----
        # mt8[p, o, j] = sum_i g[i, o*128+p] xkv[i, j], scaled to 32*m.
        # Four sub-phases of 4 PSUM groups; contraction k-pairs stream in
        # DMA-arrival order, and each sub-phase's casts (split across both
        # elementwise engines, ~680ns apiece) hide under the next one.
        mt8_sb = sb.tile([P, N_IB, NK], F8, tag="mt8", name="mt8_sb")
        for jc in range(NK // RC):
            for oh in range(2):
                accs = [(oh * 4 + i, ps.tile([P, RC], F32, tag="mm",
                                             name="acc_mt"))
                        for i in range(4)]
                kv8_h = kv8a_sb if jc == 0 else kv8b_sb
                for k in range(N_KP):
                    for o, a in accs:
                        nc.tensor.matmul(
                            a, lhsT=g8_sb[:, 2 * k:2 * k + 2, ts(o, P)],
                            rhs=kv8_h[:, 2 * k:2 * k + 2, :],
                            start=(k == 0), stop=(k == N_KP - 1),
                            perf_mode=DR)
                        if k == N_KP - 1:
                            if o % 2 == 0:
                                nc.vector.tensor_copy(
                                    mt8_sb[:, o, ts(jc, RC)], a)
                            else:
                                nc.scalar.copy(mt8_sb[:, o, ts(jc, RC)], a)

        # ---- V projection (fp16) ----
        # dc-major batches: the dc=0 half only needs wva, which arrives
        # a full load-phase before wvb
        v_sb = [sb.tile([P, D], F16, tag=f"vj{j}", name=f"vj{j}")
                for j in range(N_KT)]
        groups = [(j, dc) for dc in range(D // RC) for j in range(N_KT)]
        for gb in range(0, len(groups), 4):
            batch = groups[gb:gb + 4]
            accs = [ps.tile([P, RC], F32, tag="mm", name="acc_v")
                    for _ in batch]
            for i in range(N_IB):
                for a, (j, dc) in zip(accs, batch):
                    nc.tensor.matmul(a, lhsT=kv16_at(i)[:, ts(j, P)],
                                     rhs=wv_at(i, dc),
                                     start=(i == 0), stop=(i == N_IB - 1))
            for n, (a, (j, dc)) in enumerate(zip(accs, batch)):
                if n % 2 == 0:
                    nc.vector.tensor_copy(v_sb[j][:, ts(dc, RC)], a)
                else:
                    nc.scalar.copy(v_sb[j][:, ts(dc, RC)], a)

        # ---- attention scores (fp8 DoubleRow) ----
        # ST is emitted kt-major so the stationary MT block is reused by
        # consecutive matmuls across query chunks.
        pt_tiles = {}

        def kept_rcs(kt):
            return [rc for rc in range(N_RC) if kt in _kept_kts(rc)]

        def emit_st(kt):
            rcs = kept_rcs(kt)
            accs = {rc: ps.tile([P, RC], F32, tag="mm", name="acc_st")
                    for rc in rcs}
            for k in range(N_KP):
                for rc in rcs:
                    qo = _trim(rc, kt)
                    nc.tensor.matmul(
                        accs[rc][:, qo:RC],
                        lhsT=mt8_sb[:, 2 * k:2 * k + 2, ts(kt, P)],
                        rhs=xt8_sb[:, 2 * k:2 * k + 2,
                                   rc * RC + qo:(rc + 1) * RC],
                        start=(k == 0), stop=(k == N_KP - 1), perf_mode=DR)
            for rc in rcs:
                qo = _trim(rc, kt)
                pt = pts.tile([P, RC], F16, tag=f"pt{kt}_{rc}",
                              name=f"pt{kt}_{rc}")
                nc.scalar.activation(pt[:, qo:RC], accs[rc][:, qo:RC],
                                     mybir.ActivationFunctionType.Exp,
                                     bias=ebias, scale=EXP_SCALE)
                base = _mask_base(rc, kt) + qo
                if base < P - 1:  # tile straddles the causal diagonal
                    nc.gpsimd.affine_select(
                        out=pt[:, qo:RC], in_=pt[:, qo:RC],
                        compare_op=mybir.AluOpType.is_ge, fill=0.0,
                        base=base, channel_multiplier=-1,
                        pattern=[[1, RC - qo]])
                pt_tiles[(kt, rc)] = pt

        # ---- PV (fp16) ----
        # PT sub-blocks stationary, V moving; output lands in natural
        # [query, d] orientation; the row-sum l is an extra N=1 matmul on
        # an already-loaded stationary. Numerators ship fp16.
        def emit_pv(rc, rsub, final=False):
            # the trimmed sub-block (kt == 2rc+1, rsub < 2) is all-zero
            kts = [kt for kt in _kept_kts(rc)
                   if not (rsub < _trim(rc, kt) // P)]
            last = len(kts) - 1
            pos = [ps.tile([P, RC], F32, tag="mm", name="acc_pv")
                   for _ in range(D // RC)]
            pl = ps.tile([P, 1], F32, tag="mm", name="acc_l")
            for n, kt in enumerate(kts):
                lhs = pt_tiles[(kt, rc)][:, ts(rsub, P)]
                for dc, po in enumerate(pos):
                    nc.tensor.matmul(po, lhsT=lhs,
                                     rhs=v_sb[kt][:, ts(dc, RC)],
                                     start=(n == 0), stop=(n == last))
                nc.tensor.matmul(pl, lhsT=lhs, rhs=ones,
                                 start=(n == 0), stop=(n == last))
            # NOTE: do NOT widen ot rows past 1024 f16 cols - a 2052-byte
            # row defeats DMA burst aggregation (one packet per row,
            # ~75GB/s instead of ~200) and adds ~20us of output tail.
            idx = rc * 4 + rsub
            nc.vector.tensor_copy(l_sb[:, idx:idx + 1], pl)
            o_sb = outp.tile([P, D], F16, tag="osb", name="osb")
            if final:
                # the ls trigger only needs the l copy; emitting it here
                # lets the sync queue process it under the cast window
                nc.sync.dma_start(out=ls, in_=l_sb)
            nc.vector.tensor_copy(o_sb[:, ts(0, RC)], pos[0])
            nc.scalar.copy(o_sb[:, ts(1, RC)], pos[1])
            row = rc * RC + rsub * P
            if final:
                # tail-only split: a [128, D] transfer is 128 per-partition
                # packets on ONE queue (~3.5us); halves ride two queues
                nc.sync.dma_start(out=ot[row:row + 64, :], in_=o_sb[0:64, :])
                nc.sync.dma_start(out=ot[row + 64:row + P, :],
                                  in_=o_sb[64:P, :])
            else:
                nc.sync.dma_start(out=ot[row:row + P, :], in_=o_sb)

        for kt in range(N_KT):
            emit_st(kt)
        # pair thin (rc 0/1) with thick (rc 2/3) rsubs so the elementwise
        # engines' output casts always hide under PE compute, and end on
        # the deepest chunk (3,3) so the kernel tail is one cast + DMA
        # NOTE: gpsimd.dma_start (SWDGE) costs ~43ns on EVERY matmul in the
        # module - never add a dynamic DMA queue to this kernel.
        # Thick rsubs lead each pair (the cast/DMA chain gets a head
        # start over the thin ones) and the kernel ends on (3,3).
        for rc_pair in ((2, 0), (1, 3)):
            for rsub in range(RC // P):
                for rc in rc_pair:
                    emit_pv(rc, rsub, final=(rc == 3 and rsub == 3))


_NC_CACHE = {}


def _get_nc():
    if "nc" not in _NC_CACHE:
        nc = bacc.Bacc("TRN2", target_bir_lowering=False, debug=False,
                       enable_asserts=False, num_devices=N_CORES)
        xt8 = nc.dram_tensor("xt8", [P, N_IB, NQ], F8, kind="ExternalInput").ap()
        kv8a = nc.dram_tensor("kv8a", [P, N_IB, RC], F8,
                              kind="ExternalInput").ap()
        kv8b = nc.dram_tensor("kv8b", [P, N_IB, RC], F8,
                              kind="ExternalInput").ap()
        g8 = nc.dram_tensor("g8", [P, N_IB, D], F8, kind="ExternalInput").ap()
        kv16 = nc.dram_tensor("kv16", [P, N_IB, NK], F16,
                              kind="ExternalInput").ap()
        wva = nc.dram_tensor("wva", [P, N_IB, RC], F16,
                             kind="ExternalInput").ap()
        wvb = nc.dram_tensor("wvb", [P, N_IB, RC], F16,
                             kind="ExternalInput").ap()
        ot = nc.dram_tensor("ot", [NQ, D], F16, kind="ExternalOutput").ap()
        ls = nc.dram_tensor("ls", [P, N_RC * 4], F32, kind="ExternalOutput").ap()
        with tile.TileContext(nc) as tc:
            _emit(nc, tc, xt8, kv8a, kv8b, g8, kv16, wva, wvb, ot, ls)
        nc.compile()
        _NC_CACHE["nc"] = nc
    return _NC_CACHE["nc"]


def _blk(a, width):
    # [D, width] row-major -> [128, 8, width] (dim1 = 128-row block)
    return np.ascontiguousarray(
        a.reshape(N_IB, P, width).transpose(1, 0, 2))


def _f8(a, scale):
    return np.asarray(np.clip(a * scale, -240.0, 240.0),
                      dtype=ml_dtypes.float8_e4m3)


def make_in_maps(x, w_query, w_key, w_value):
    wq32 = np.asarray(w_query, dtype=np.float32)
    wk32 = np.asarray(w_key, dtype=np.float32)
    # fold the Q and K projections: scores = x_kv (Wk^T Wq) x^T
    g_np = np.ascontiguousarray(wk32.T @ wq32)
    g8_np = _blk(_f8(g_np, G_SCALE), D)
    wvt_np = np.ascontiguousarray(np.asarray(w_value).T).astype(np.float16)
    wva_np = _blk(np.ascontiguousarray(wvt_np[:, 0:RC]), RC)
    wvb_np = _blk(np.ascontiguousarray(wvt_np[:, RC:D]), RC)
    kv_cols = (np.arange(NK) // STRIPE) * (2 * STRIPE) + np.arange(NK) % STRIPE
    in_maps = []
    for c in range(N_CORES):
        b, eta = c // 2, c % 2
        rows = (np.arange(NQ) + eta * STRIPE) % S  # cols past S wrap to junk
        xt_np = np.ascontiguousarray(np.asarray(x)[b, rows].T)  # [D, NQ] f32
        xkv_np = xt_np[:, kv_cols]                              # [D, NK] f32
        kv8_np = _f8(xkv_np, X_SCALE)
        in_maps.append({
            "xt8": _blk(_f8(xt_np, X_SCALE), NQ),
            "kv8a": _blk(np.ascontiguousarray(kv8_np[:, 0:RC]), RC),
            "kv8b": _blk(np.ascontiguousarray(kv8_np[:, RC:NK]), RC),
            "g8": g8_np,
            "kv16": _blk(xkv_np.astype(np.float16), NK),
            "wva": wva_np,
            "wvb": wvb_np,
        })
    return in_maps


def merge_outputs(results):
    num = np.zeros((B, S, D), np.float32)
    den = np.zeros((B, S), np.float32)
    for c in range(N_CORES):
        b, eta = c // 2, c % 2
        # ot is the fp16 numerator, ls the denominator (both carry the
        # common 1/4 prescale from EXP_BIAS; the ratio is exact)
        otc = np.asarray(results[c]["ot"]).astype(np.float32)  # [NQ, D]
        # ls[p, col] holds l for query col c = col*128 + p
        lc = np.asarray(results[c]["ls"]).T.reshape(NQ)
        beta = eta * STRIPE
        nvalid = S - beta
        num[b, beta:] += otc[:nvalid]
        den[b, beta:] += lc[:nvalid]
    return (num / den[:, :, None]).astype(np.float32)


def kernel(x, w_query, w_key, w_value, _trace=False):
    nc = _get_nc()
    in_maps = make_in_maps(x, w_query, w_key, w_value)
    res = bass_utils.run_bass_kernel_spmd(
        nc, in_maps, core_ids=list(range(N_CORES)), trace=_trace)
    out = merge_outputs(res.results)
    if _trace:
        kernel.last_result = res
    return out
